# revision 14
# baseline (speedup 1.0000x reference)
"""Two-layer GAT (PyG GATConv semantics, eval mode) on 8 Trainium2 NeuronCores.

Strategy (dst-sharded, edge-block matmul segment-sum):
  - Host: add self-loops, permute nodes so every 128-node "block" has an
    approximately equal number of incoming edges (snake packing by in-degree),
    assign 49 blocks to each of the 8 cores, group edges by dst block, split
    each block's edges by src < 32768 (int16 gather-index limit), pad each
    group to a fixed tile count.
  - Device, per core (SPMD, one compiled program):
      Phase A: xp_aug = x @ [W1 | W1 a_src | W1 a_dst] for ALL nodes
               (replicated), stored to an HBM gather table of 1280B rows.
      Phase B1: per dst block: dma_gather fused feature+score rows by src,
               dma_gather dst scores from an own-shard table, build per-tile
               one-hot M^T via iota-compare, fold exp(LeakyReLU(e)) into the
               rhs, and accumulate [aggregated messages | softmax denom] in
               PSUM with the tensor engine. Softmax max-subtraction is skipped
               (scores are O(10), exp is safe in fp32).
      Phase C: xp2_aug = h @ [W2 | W2 a2_src | W2 a2_dst] for own nodes,
               AllGather across the 8 cores.
      Phase B2: same edge machinery for layer 2; write z shard.
  - Host: concat shards, invert the node permutation.
"""

import os
import sys
from dataclasses import dataclass

import numpy as np

for _p in ("/opt/trn_rl_repo", "/root/.axon_site/_ro/trn_rl_repo"):
    if os.path.isdir(_p) and _p not in sys.path:
        sys.path.append(_p)

import concourse.bacc as bacc
import concourse.bass as bass
import concourse.mybir as mybir
import concourse.tile as tile
from concourse import bass_utils

F32 = mybir.dt.float32
I16 = mybir.dt.int16

NEG_SLOPE = 0.2
EXP_CLAMP = 30.0


class _Trunc(Exception):
    """Phase-truncation sentinel for KPHASES debugging builds."""


@dataclass(frozen=True)
class GATCfg:
    n_cores: int
    n_pad: int        # padded node count (blocks_total * 128)
    npc: int          # nodes per core
    bpc: int          # blocks per core
    lo_rows: int      # src ids < lo_rows go through the "lo" gather table
    t_lo: int         # tiles of 128 lo-src edges per block
    t_hi: int         # tiles of 128 hi-src edges per block
    in_c: int         # input channels (128)
    hc: int           # heads * hid (256)
    heads: int        # 4
    hid: int          # 64
    out_c: int        # 64
    row1: int         # layer-1 table row width in floats (hc + 64)
    ag_padded: bool   # AllGather into padded 128-wide rows (fallback path)

    @property
    def t_b(self):
        return self.t_lo + self.t_hi


def _wrap_idx(arr):
    """dma_gather index layout: linear i -> (partition i%16, col i//16),
    replicated across the 8 Q7 cores (16-partition pattern tiled to 128)."""
    assert arr.size % 16 == 0
    w = arr.reshape(-1, 16).T  # [16, n/16]
    return np.tile(w, (8, 1))  # [128, n/16]


def prep(x, edge_index, W1, a1_src, a1_dst, b1, W2, a2_src, a2_dst, b2,
         n_cores=8, lo_rows_cap=32768):
    N, IN_C = x.shape
    HEADS, HID = a1_src.shape
    HC = HEADS * HID
    OUT_C = W2.shape[1]

    blk_per_core = -(-N // (128 * n_cores))
    npc = blk_per_core * 128
    n_pad = npc * n_cores
    blocks_total = n_pad // 128
    lo_rows = min(lo_rows_cap, n_pad)

    src = np.asarray(edge_index[0], dtype=np.int64)
    dst = np.asarray(edge_index[1], dtype=np.int64)

    # in-degree incl. self-loop, over padded node set
    deg = np.bincount(dst, minlength=n_pad).astype(np.int64) + 1

    # snake-pack nodes into blocks by descending degree -> balanced block loads
    order = np.argsort(-deg, kind="stable")
    rounds = np.arange(n_pad) // blocks_total
    pos = np.arange(n_pad) % blocks_total
    blk_of_sorted = np.where(rounds % 2 == 0, pos, blocks_total - 1 - pos)
    slot_of_sorted = rounds
    pid_of = np.empty(n_pad, dtype=np.int64)
    pid_of[order] = blk_of_sorted * 128 + slot_of_sorted

    # all edges incl. self-loops for every (padded) node, in permuted space
    ps = np.concatenate([pid_of[src], np.arange(n_pad)])
    pd = np.concatenate([pid_of[dst], np.arange(n_pad)])
    pd_blk = pd >> 7

    is_lo = ps < lo_rows
    # group edges by (block, hi/lo): sort by block*2 + (1-is_lo)
    gkey = pd_blk * 2 + (~is_lo).astype(np.int64)
    eorder = np.argsort(gkey, kind="stable")
    ps_s, pd_s, key_s = ps[eorder], pd[eorder], gkey[eorder]

    cnt = np.bincount(gkey, minlength=blocks_total * 2)
    cnt_lo = cnt[0::2]
    cnt_hi = cnt[1::2]
    t_lo = int(-(-cnt_lo.max() // 128)) if cnt_lo.max() > 0 else 0
    t_hi = int(-(-cnt_hi.max() // 128)) if cnt_hi.max() > 0 else 0
    if t_hi == 0 and lo_rows < n_pad:
        t_hi = 1
    t_b = t_lo + t_hi
    bpc = blk_per_core

    # per-block slot arrays
    slots = blocks_total * t_b * 128
    slot_ps = np.zeros(slots, dtype=np.int64)          # gather idx (pad 0)
    slot_rel = np.full(slots, -1.0, dtype=np.float32)  # dst_rel (pad -1)
    slot_pd = np.zeros(slots, dtype=np.int64)          # dst id   (pad 0)

    ends = np.cumsum(cnt)
    starts = ends - cnt
    # positions of each group's edges within the block's slot array
    grp = key_s
    within = np.arange(len(ps_s)) - starts[grp]
    base = (grp >> 1) * (t_b * 128) + np.where(grp % 2 == 0, 0, t_lo * 128)
    slot_idx = base + within
    slot_ps[slot_idx] = ps_s
    slot_rel[slot_idx] = (pd_s & 127).astype(np.float32)
    slot_pd[slot_idx] = pd_s
    # padding dst ids: keep 0 -> but idx_dst is relative to the OWN core's
    # shard; pad slots use the core's first node (rel id 0) which is valid.

    slot_ps = slot_ps.reshape(n_cores, bpc, t_b * 128)
    slot_rel = slot_rel.reshape(n_cores, bpc, t_b * 128)
    slot_pd = slot_pd.reshape(n_cores, bpc, t_b * 128)

    hi_elems = 0 if t_hi == 0 else 1
    cfg = GATCfg(n_cores=n_cores, n_pad=n_pad, npc=npc, bpc=bpc,
                 lo_rows=lo_rows, t_lo=t_lo, t_hi=t_hi, in_c=IN_C, hc=HC,
                 heads=HEADS, hid=HID, out_c=OUT_C, row1=HC + 64,
                 ag_padded=True)

    # ---- shared (replicated) tensors ----
    xT = np.zeros((IN_C, n_pad), dtype=np.float32)
    xT[:, pid_of[:N]] = np.asarray(x, dtype=np.float32).T

    W1 = np.asarray(W1, np.float32)
    w1s = np.stack([W1[:, h * HID:(h + 1) * HID] @ np.asarray(a1_src, np.float32)[h]
                    for h in range(HEADS)], axis=1)          # [IN_C, H]
    w1d = np.stack([W1[:, h * HID:(h + 1) * HID] @ np.asarray(a1_dst, np.float32)[h]
                    for h in range(HEADS)], axis=1)
    W1a = np.concatenate([W1, w1s, w1d], axis=1)             # [IN_C, HC+8]
    W1a_pad = np.zeros((IN_C, HC + 16), dtype=np.float32)
    W1a_pad[:, :HC + 8] = W1a

    W2 = np.asarray(W2, np.float32)
    w2s = (W2 @ np.asarray(a2_src, np.float32)[0])[:, None]  # [HC, 1]
    w2d = (W2 @ np.asarray(a2_dst, np.float32)[0])[:, None]
    W2a = np.concatenate([W2, w2s, w2d], axis=1)             # [HC, OUT_C+2]
    c2 = OUT_C + 2
    W2s = np.zeros((128, (HC // 128) * c2), dtype=np.float32)
    for j in range(HC // 128):
        W2s[:, j * c2:(j + 1) * c2] = W2a[j * 128:(j + 1) * 128]

    B1 = np.tile(np.asarray(b1, np.float32)[None, :], (128, 1))
    B2 = np.tile(np.asarray(b2, np.float32)[None, :], (128, 1))
    IOTA = np.tile(np.arange(128, dtype=np.float32)[None, :], (128, 1))
    IDN = np.eye(128, dtype=np.float32)

    in_maps = []
    for c in range(n_cores):
        lo_parts, hi_parts, dst_parts = [], [], []
        for b in range(bpc):
            s_ps = slot_ps[c, b]
            s_pd = slot_pd[c, b]
            lo_parts.append(_wrap_idx(s_ps[:t_lo * 128].astype(np.int16)))
            if t_hi:
                hi_parts.append(_wrap_idx(
                    (s_ps[t_lo * 128:] - lo_rows).clip(min=0).astype(np.int16)))
            dst_parts.append(_wrap_idx((s_pd - c * npc).clip(0, npc - 1)
                                       .astype(np.int16)))
        # dst_rel matrix: [128 lanes, bpc*t_b tiles]
        rel = slot_rel[c].reshape(bpc * t_b, 128).T.copy()
        m = {
            "xT": xT, "W1a": W1a_pad, "W2s": W2s, "B1": B1, "B2": B2,
            "IOTA": IOTA, "IDN": IDN,
            "idxlo": np.concatenate(lo_parts, axis=1).astype(np.int16),
            "idxdst": np.concatenate(dst_parts, axis=1).astype(np.int16),
            "dstrel": np.ascontiguousarray(rel),
        }
        if t_hi:
            m["idxhi"] = np.concatenate(hi_parts, axis=1).astype(np.int16)
        in_maps.append(m)

    return cfg, in_maps, pid_of[:N]


def build(cfg: GATCfg):
    level = {"A": 0, "B1": 1, "C": 2, "AG": 3, "full": 4}[
        os.environ.get("KPHASES", "full")]
    P = 128
    HC, H, HID, OC = cfg.hc, cfg.heads, cfg.hid, cfg.out_c
    R1 = cfg.row1
    C2 = OC + 2
    T_LO, T_HI, T_B = cfg.t_lo, cfg.t_hi, cfg.t_b
    BPC, NPC, NPAD = cfg.bpc, cfg.npc, cfg.n_pad
    LO = cfg.lo_rows
    NBLK = NPAD // P
    R2 = 128  # layer-2 table row width (floats)

    nc = bacc.Bacc("TRN2", target_bir_lowering=False, debug=False,
                   num_devices=cfg.n_cores)

    xT_t = nc.dram_tensor("xT", [cfg.in_c, NPAD], F32, kind="ExternalInput")
    W1a_t = nc.dram_tensor("W1a", [cfg.in_c, HC + 16], F32, kind="ExternalInput")
    W2s_t = nc.dram_tensor("W2s", [P, (HC // P) * C2], F32, kind="ExternalInput")
    B1_t = nc.dram_tensor("B1", [P, HC], F32, kind="ExternalInput")
    B2_t = nc.dram_tensor("B2", [P, OC], F32, kind="ExternalInput")
    IOTA_t = nc.dram_tensor("IOTA", [P, P], F32, kind="ExternalInput")
    IDN_t = nc.dram_tensor("IDN", [P, P], F32, kind="ExternalInput")
    idxlo_t = nc.dram_tensor("idxlo", [P, BPC * T_LO * 8], I16, kind="ExternalInput")
    idxhi_t = (nc.dram_tensor("idxhi", [P, BPC * T_HI * 8], I16, kind="ExternalInput")
               if T_HI else None)
    idxdst_t = nc.dram_tensor("idxdst", [P, BPC * T_B * 8], I16, kind="ExternalInput")
    dstrel_t = nc.dram_tensor("dstrel", [P, BPC * T_B], F32, kind="ExternalInput")
    z_t = nc.dram_tensor("z", [NPC, OC], F32, kind="ExternalOutput")
    hdump_t = (nc.dram_tensor("hdump", [NPC, HC], F32, kind="ExternalOutput")
               if os.environ.get("KDEBUG") else None)
    pdump_t = (nc.dram_tensor("pdump", [NPC, HC + H], F32, kind="ExternalOutput")
               if os.environ.get("KDEBUG") else None)

    if True:
      with tile.TileContext(nc) as tc:
        with tc.tile_pool(name="dram", bufs=1, space="DRAM") as dram:
            xp_tab = dram.tile([NPAD + 1, R1], F32)
            al_own = dram.tile([NPC, 64], F32)
            xp2_own = dram.tile([NPC, R2], F32)
            xp2_tab = dram.tile([NPAD, R2], F32)
            al2_own = dram.tile([NPC, 64], F32)

            with tc.tile_pool(name="consts", bufs=1) as consts:
                w1a = consts.tile([P, HC + 16], F32)
                w2s = consts.tile([P, (HC // P) * C2], F32)
                b1t = consts.tile([P, HC], F32)
                b2t = consts.tile([P, OC], F32)
                iota = consts.tile([P, P], F32)
                idn = consts.tile([P, P], F32)
                nc.sync.dma_start(out=w1a[:], in_=W1a_t.ap())
                nc.sync.dma_start(out=w2s[:], in_=W2s_t.ap())
                nc.sync.dma_start(out=b1t[:], in_=B1_t.ap())
                nc.sync.dma_start(out=b2t[:], in_=B2_t.ap())
                nc.sync.dma_start(out=iota[:], in_=IOTA_t.ap())
                nc.sync.dma_start(out=idn[:], in_=IDN_t.ap())

                idxlo = consts.tile([P, BPC * T_LO * 8], I16)
                nc.sync.dma_start(out=idxlo[:], in_=idxlo_t.ap())
                if T_HI:
                    idxhi = consts.tile([P, BPC * T_HI * 8], I16)
                    nc.sync.dma_start(out=idxhi[:], in_=idxhi_t.ap())
                idxdst = consts.tile([P, BPC * T_B * 8], I16)
                nc.sync.dma_start(out=idxdst[:], in_=idxdst_t.ap())
                dstrel = consts.tile([P, BPC * T_B], F32)
                nc.sync.dma_start(out=dstrel[:], in_=dstrel_t.ap())

                h_sb = consts.tile([P, BPC * HC], F32)  # layer-1 out, own nodes

                # ---------------- Phase A ----------------
                CH = min(8, NBLK)  # node tiles per xT load
                with tc.tile_pool(name="pa_x", bufs=2) as pa_x, \
                     tc.tile_pool(name="pa_ps", bufs=2, space="PSUM") as pa_ps, \
                     tc.tile_pool(name="pa_o", bufs=3) as pa_o:
                    for ch0 in range(0, NBLK, CH):
                        cw = min(CH, NBLK - ch0)
                        xt = pa_x.tile([P, CH * P], F32, tag="xt")
                        nc.sync.dma_start(
                            out=xt[:, 0:cw * P],
                            in_=xT_t.ap()[:, ch0 * P:(ch0 + cw) * P])
                        for j in range(cw):
                            t = ch0 + j
                            ps = pa_ps.tile([P, HC + 16], F32, tag="paps")
                            nc.tensor.matmul(out=ps[:], lhsT=xt[:, j * P:(j + 1) * P],
                                             rhs=w1a[:], start=True, stop=True)
                            ot = pa_o.tile([P, HC + 8], F32, tag="pao")
                            nc.any.tensor_copy(out=ot[:], in_=ps[:, 0:HC + 8])
                            nc.sync.dma_start(
                                out=xp_tab[t * P:(t + 1) * P, 0:HC + 8], in_=ot[:])

                # own-shard score table: al_own[:, 0:8] = xp_tab[c*NPC:.., HC:HC+8]
                pid = nc.gpsimd.partition_id()
                row0 = pid * NPC
                nc.gpsimd.dma_start(
                    out=al_own[:, 0:8],
                    in_=xp_tab[bass.ds(row0, NPC), HC:HC + 8])

                # ---------------- Phase B1 ----------------
                with tc.tile_pool(name="b1_sx", bufs=2) as sxp, \
                     tc.tile_pool(name="b1_sd", bufs=2) as sdp, \
                     tc.tile_pool(name="b1_mt", bufs=4) as mtp, \
                     tc.tile_pool(name="b1_rhs", bufs=4) as rhp, \
                     tc.tile_pool(name="b1_sm", bufs=8) as smp, \
                     tc.tile_pool(name="b1_ps", bufs=2, space="PSUM") as psp, \
                     tc.tile_pool(name="b1_hw", bufs=3) as hwp:
                    if level < 2:
                        nc.vector.memset(h_sb[:], 0.0)
                    for b in range(BPC if level >= 1 else 0):
                        sx = sxp.tile([P, T_B, R1], F32, tag="sx")
                        sd = sdp.tile([P, T_B, 64], F32, tag="sd")
                        nc.gpsimd.dma_gather(
                            out_ap=sx[:, 0:T_LO, :],
                            in_ap=xp_tab[0:LO, :],
                            idxs_ap=idxlo[:, b * T_LO * 8:(b + 1) * T_LO * 8],
                            num_idxs=T_LO * P, num_idxs_reg=T_LO * P,
                            elem_size=R1, single_packet=False)
                        if T_HI:
                            nc.gpsimd.dma_gather(
                                out_ap=sx[:, T_LO:T_B, :],
                                in_ap=xp_tab[LO:NPAD, :],
                                idxs_ap=idxhi[:, b * T_HI * 8:(b + 1) * T_HI * 8],
                                num_idxs=T_HI * P, num_idxs_reg=T_HI * P,
                                elem_size=R1, single_packet=False)
                        nc.gpsimd.dma_gather(
                            out_ap=sd[:],
                            in_ap=al_own[:],
                            idxs_ap=idxdst[:, b * T_B * 8:(b + 1) * T_B * 8],
                            num_idxs=T_B * P, num_idxs_reg=T_B * P,
                            elem_size=64, single_packet=False)
                        psb = psp.tile([P, HC + H], F32, tag="psb")
                        for t in range(T_B):
                            gt = b * T_B + t
                            X = sx[:, t, :]
                            mt = mtp.tile([P, P], F32, tag="mt")
                            nc.vector.tensor_scalar(
                                out=mt[:], in0=iota[:],
                                scalar1=dstrel[:, gt:gt + 1], scalar2=None,
                                op0=mybir.AluOpType.is_equal)
                            z4 = smp.tile([P, H], F32, tag="z4")
                            nc.vector.tensor_tensor(
                                out=z4[:], in0=X[:, HC:HC + H],
                                in1=sd[:, t, 4:4 + H], op=mybir.AluOpType.add)
                            zl = smp.tile([P, H], F32, tag="zl")
                            nc.vector.tensor_scalar(
                                out=zl[:], in0=z4[:], scalar1=NEG_SLOPE,
                                scalar2=EXP_CLAMP, op0=mybir.AluOpType.mult,
                                op1=mybir.AluOpType.min)
                            ze = smp.tile([P, H], F32, tag="ze")
                            nc.vector.tensor_scalar(
                                out=ze[:], in0=z4[:], scalar1=EXP_CLAMP,
                                scalar2=None, op0=mybir.AluOpType.min)
                            rt = rhp.tile([P, HC + H], F32, tag="rt")
                            nc.vector.tensor_tensor(
                                out=rt[:, HC:HC + H], in0=ze[:], in1=zl[:],
                                op=mybir.AluOpType.max)
                            nc.scalar.activation(
                                out=rt[:, HC:HC + H], in_=rt[:, HC:HC + H],
                                func=mybir.ActivationFunctionType.Exp)
                            for h in range(H):
                                nc.scalar.mul(
                                    out=rt[:, h * HID:(h + 1) * HID],
                                    in_=X[:, h * HID:(h + 1) * HID],
                                    mul=rt[:, HC + h:HC + h + 1])
                            nc.tensor.matmul(out=psb[:], lhsT=mt[:], rhs=rt[:],
                                             start=(t == 0), stop=(t == T_B - 1))
                        # block epilogue: h = ELU(psum/denom + b1)
                        if pdump_t is not None:
                            pd_sb = hwp.tile([P, HC + H], F32, tag="pdsb")
                            nc.any.tensor_copy(out=pd_sb[:], in_=psb[:])
                            nc.sync.dma_start(
                                out=pdump_t.ap()[b * P:(b + 1) * P, :],
                                in_=pd_sb[:])
                        rec = smp.tile([P, H], F32, tag="rec")
                        nc.vector.reciprocal(out=rec[:], in_=psb[:, HC:HC + H])
                        hb = hwp.tile([P, HC], F32, tag="hb")
                        for h in range(H):
                            nc.scalar.mul(out=hb[:, h * HID:(h + 1) * HID],
                                          in_=psb[:, h * HID:(h + 1) * HID],
                                          mul=rec[:, h:h + 1])
                        nc.vector.tensor_tensor(out=hb[:], in0=hb[:], in1=b1t[:],
                                                op=mybir.AluOpType.add)
                        tn = hwp.tile([P, HC], F32, tag="tn")
                        nc.vector.tensor_scalar(
                            out=tn[:], in0=hb[:], scalar1=0.0, scalar2=None,
                            op0=mybir.AluOpType.min)
                        nc.scalar.activation(out=tn[:], in_=tn[:],
                                             func=mybir.ActivationFunctionType.Exp)
                        tp = hwp.tile([P, HC], F32, tag="tp")
                        nc.vector.tensor_scalar(
                            out=tp[:], in0=hb[:], scalar1=0.0, scalar2=None,
                            op0=mybir.AluOpType.max)
                        nc.vector.tensor_tensor(out=tn[:], in0=tn[:], in1=tp[:],
                                                op=mybir.AluOpType.add)
                        nc.vector.tensor_scalar(
                            out=h_sb[:, b * HC:(b + 1) * HC], in0=tn[:],
                            scalar1=-1.0, scalar2=None, op0=mybir.AluOpType.add)
                        if hdump_t is not None:
                            nc.sync.dma_start(
                                out=hdump_t.ap()[b * P:(b + 1) * P, :],
                                in_=h_sb[:, b * HC:(b + 1) * HC])

                # ---------------- Phase C ----------------
                with tc.tile_pool(name="c_tp", bufs=2, space="PSUM") as ctp, \
                     tc.tile_pool(name="c_ps", bufs=2, space="PSUM") as cps, \
                     tc.tile_pool(name="c_hT", bufs=3) as chp, \
                     tc.tile_pool(name="c_o", bufs=3) as cop:
                    for b in range(BPC if level >= 2 else 0):
                        p2 = cps.tile([P, C2], F32, tag="p2")
                        for j in range(HC // P):
                            pt = ctp.tile([P, P], F32, tag="pt")
                            nc.tensor.transpose(
                                out=pt[:],
                                in_=h_sb[:, b * HC + j * P: b * HC + (j + 1) * P],
                                identity=idn[:])
                            hT = chp.tile([P, P], F32, tag="hT")
                            nc.any.tensor_copy(out=hT[:], in_=pt[:])
                            nc.tensor.matmul(out=p2[:], lhsT=hT[:],
                                             rhs=w2s[:, j * C2:(j + 1) * C2],
                                             start=(j == 0), stop=(j == HC // P - 1))
                        o2 = cop.tile([P, R2], F32, tag="o2")
                        nc.vector.memset(o2[:, C2:R2], 0.0)
                        nc.any.tensor_copy(out=o2[:, 0:C2], in_=p2[:])
                        nc.sync.dma_start(out=xp2_own[b * P:(b + 1) * P, :],
                                          in_=o2[:])

                if level >= 3:
                    nc.gpsimd.collective_compute(
                        "AllGather", mybir.AluOpType.bypass,
                        ins=[xp2_own.opt()],
                        outs=[xp2_tab.opt()],
                        replica_groups=[list(range(cfg.n_cores))])

                    nc.gpsimd.dma_start(
                        out=al2_own[:, 0:2],
                        in_=xp2_tab[bass.ds(row0, NPC), OC:OC + 2])

                # ---------------- Phase B2 ----------------
                with tc.tile_pool(name="b2_sx", bufs=2) as sxp2, \
                     tc.tile_pool(name="b2_sd", bufs=2) as sdp2, \
                     tc.tile_pool(name="b2_mt", bufs=4) as mtp2, \
                     tc.tile_pool(name="b2_rhs", bufs=4) as rhp2, \
                     tc.tile_pool(name="b2_sm", bufs=8) as smp2, \
                     tc.tile_pool(name="b2_ps", bufs=2, space="PSUM") as psp2, \
                     tc.tile_pool(name="b2_z", bufs=3) as zp:
                    for b in range(BPC if level >= 4 else 0):
                        sx = sxp2.tile([P, T_B, R2], F32, tag="sx2")
                        sd = sdp2.tile([P, T_B, 64], F32, tag="sd2")
                        nc.gpsimd.dma_gather(
                            out_ap=sx[:, 0:T_LO, :],
                            in_ap=xp2_tab[0:LO, :],
                            idxs_ap=idxlo[:, b * T_LO * 8:(b + 1) * T_LO * 8],
                            num_idxs=T_LO * P, num_idxs_reg=T_LO * P,
                            elem_size=R2, single_packet=False)
                        if T_HI:
                            nc.gpsimd.dma_gather(
                                out_ap=sx[:, T_LO:T_B, :],
                                in_ap=xp2_tab[LO:NPAD, :],
                                idxs_ap=idxhi[:, b * T_HI * 8:(b + 1) * T_HI * 8],
                                num_idxs=T_HI * P, num_idxs_reg=T_HI * P,
                                elem_size=R2, single_packet=False)
                        nc.gpsimd.dma_gather(
                            out_ap=sd[:],
                            in_ap=al2_own[:],
                            idxs_ap=idxdst[:, b * T_B * 8:(b + 1) * T_B * 8],
                            num_idxs=T_B * P, num_idxs_reg=T_B * P,
                            elem_size=64, single_packet=False)
                        psb = psp2.tile([P, OC + 1], F32, tag="psb2")
                        for t in range(T_B):
                            gt = b * T_B + t
                            X = sx[:, t, :]
                            mt = mtp2.tile([P, P], F32, tag="mt2")
                            nc.vector.tensor_scalar(
                                out=mt[:], in0=iota[:],
                                scalar1=dstrel[:, gt:gt + 1], scalar2=None,
                                op0=mybir.AluOpType.is_equal)
                            z1 = smp2.tile([P, 1], F32, tag="z1")
                            nc.vector.tensor_tensor(
                                out=z1[:], in0=X[:, OC:OC + 1],
                                in1=sd[:, t, 1:2], op=mybir.AluOpType.add)
                            zl = smp2.tile([P, 1], F32, tag="zl2")
                            nc.vector.tensor_scalar(
                                out=zl[:], in0=z1[:], scalar1=NEG_SLOPE,
                                scalar2=EXP_CLAMP, op0=mybir.AluOpType.mult,
                                op1=mybir.AluOpType.min)
                            ze = smp2.tile([P, 1], F32, tag="ze2")
                            nc.vector.tensor_scalar(
                                out=ze[:], in0=z1[:], scalar1=EXP_CLAMP,
                                scalar2=None, op0=mybir.AluOpType.min)
                            rt = rhp2.tile([P, OC + 1], F32, tag="rt2")
                            nc.vector.tensor_tensor(
                                out=rt[:, OC:OC + 1], in0=ze[:], in1=zl[:],
                                op=mybir.AluOpType.max)
                            nc.scalar.activation(
                                out=rt[:, OC:OC + 1], in_=rt[:, OC:OC + 1],
                                func=mybir.ActivationFunctionType.Exp)
                            nc.scalar.mul(out=rt[:, 0:OC], in_=X[:, 0:OC],
                                          mul=rt[:, OC:OC + 1])
                            nc.tensor.matmul(out=psb[:], lhsT=mt[:], rhs=rt[:],
                                             start=(t == 0), stop=(t == T_B - 1))
                        rec = smp2.tile([P, 1], F32, tag="rec2")
                        nc.vector.reciprocal(out=rec[:], in_=psb[:, OC:OC + 1])
                        zb = zp.tile([P, OC], F32, tag="zb")
                        nc.scalar.mul(out=zb[:], in_=psb[:, 0:OC], mul=rec[:])
                        nc.vector.tensor_tensor(out=zb[:], in0=zb[:], in1=b2t[:],
                                                op=mybir.AluOpType.add)
                        nc.sync.dma_start(out=z_t.ap()[b * P:(b + 1) * P, :],
                                          in_=zb[:])

    nc.compile()
    return nc


_CACHE = {}


def _get_built(cfg):
    key = (cfg, os.environ.get("KPHASES", "full"))
    if key not in _CACHE:
        _CACHE[key] = build(cfg)
    return _CACHE[key]


class Runner:
    """Executes the compiled Bass module via PJRT/shard_map with inputs
    pre-sharded per device (no on-device resharding programs)."""

    def __init__(self, nc, n_cores):
        import jax
        from jax.sharding import Mesh, PartitionSpec, NamedSharding
        from jax.experimental.shard_map import shard_map
        from concourse import bass2jax

        bass2jax.install_neuronx_cc_hook()
        self.jax = jax
        self.nc = nc
        self.n_cores = n_cores

        pname = nc.partition_id_tensor.name if nc.partition_id_tensor else None
        in_names, out_names, out_avals = [], [], []
        for alloc in nc.m.functions[0].allocations:
            if not isinstance(alloc, mybir.MemoryLocationSet):
                continue
            name = alloc.memorylocations[0].name
            if alloc.kind == "ExternalInput":
                if name != pname:
                    in_names.append(name)
            elif alloc.kind == "ExternalOutput":
                out_names.append(name)
                out_avals.append(jax.core.ShapedArray(
                    tuple(alloc.tensor_shape), mybir.dt.np(alloc.dtype)))
        self.in_names, self.out_names, self.out_avals = in_names, out_names, out_avals
        all_in = list(in_names) + list(out_names)
        if pname is not None:
            all_in.append(pname)

        def _body(*args):
            operands = list(args)
            if pname is not None:
                operands.append(bass2jax.partition_id_tensor())
            outs = bass2jax._bass_exec_p.bind(
                *operands,
                out_avals=tuple(out_avals),
                in_names=tuple(all_in),
                out_names=tuple(out_names),
                lowering_input_output_aliases=(),
                sim_require_finite=True,
                sim_require_nnan=True,
                nc=nc,
            )
            return tuple(outs)

        self.devices = jax.devices()[:n_cores]
        self.mesh = Mesh(np.asarray(self.devices), ("core",))
        self.sh = NamedSharding(self.mesh, PartitionSpec("core"))
        nspec = (PartitionSpec("core"),)
        self.fn = jax.jit(
            shard_map(_body, mesh=self.mesh,
                      in_specs=nspec * (len(in_names) + len(out_names)),
                      out_specs=nspec * len(out_names), check_rep=False),
            keep_unused=True)
        self.dev_args = None

    def _shard(self, per_core):
        jax = self.jax
        a0 = np.asarray(per_core[0])
        gshape = (self.n_cores * a0.shape[0],) + a0.shape[1:]
        bufs = [jax.device_put(np.asarray(per_core[c]), self.devices[c])
                for c in range(self.n_cores)]
        return jax.make_array_from_single_device_arrays(gshape, self.sh, bufs)

    def set_inputs(self, in_maps):
        args = [self._shard([m[name] for m in in_maps])
                for name in self.in_names]
        for av in self.out_avals:
            z = np.zeros(av.shape, av.dtype)
            args.append(self._shard([z] * self.n_cores))
        self.dev_args = args

    def call(self):
        outs = self.fn(*self.dev_args)
        self.jax.block_until_ready(outs)
        return outs

    def run(self, in_maps):
        self.set_inputs(in_maps)
        outs = self.call()
        res = []
        for c in range(self.n_cores):
            d = {}
            for i, name in enumerate(self.out_names):
                g = np.asarray(outs[i])
                n0 = self.out_avals[i].shape[0]
                d[name] = g.reshape(self.n_cores, n0, *self.out_avals[i].shape[1:])[c]
            res.append(d)
        return res


_RUNNERS = {}


def _get_runner(cfg, nc):
    key = id(nc)
    if key not in _RUNNERS:
        _RUNNERS[key] = Runner(nc, cfg.n_cores)
    return _RUNNERS[key]


def kernel(x, edge_index, W1, a1_src, a1_dst, b1, W2, a2_src, a2_dst, b2):
    x = np.asarray(x)
    cfg, in_maps, pid_of = prep(x, edge_index, W1, a1_src, a1_dst, b1,
                                W2, a2_src, a2_dst, b2)
    nc = _get_built(cfg)
    runner = _get_runner(cfg, nc)
    results = runner.run(in_maps)
    z_full = np.concatenate([results[c]["z"] for c in range(cfg.n_cores)],
                            axis=0)
    return np.ascontiguousarray(z_full[pid_of]).astype(np.float32)


# revision 15
# speedup vs baseline: 18.7973x; 18.7973x over previous
"""Two-layer GAT (PyG GATConv semantics, eval mode) on 8 Trainium2 NeuronCores.

Strategy (dst-sharded, edge-block matmul segment-sum):
  - Host: add self-loops, permute nodes so every 128-node "block" has an
    approximately equal number of incoming edges (snake packing by in-degree),
    assign 49 blocks to each of the 8 cores, group edges by dst block, split
    each block's edges by src < 32768 (int16 gather-index limit), pad each
    group to a fixed tile count.
  - Device, per core (SPMD, one compiled program):
      Phase A: xp_aug = x @ [W1 | W1 a_src | W1 a_dst] for ALL nodes
               (replicated), stored to an HBM gather table of 1280B rows.
      Phase B1: per dst block: dma_gather fused feature+score rows by src,
               dma_gather dst scores from an own-shard table, build per-tile
               one-hot M^T via iota-compare, fold exp(LeakyReLU(e)) into the
               rhs, and accumulate [aggregated messages | softmax denom] in
               PSUM with the tensor engine. Softmax max-subtraction is skipped
               (scores are O(10), exp is safe in fp32).
      Phase C: xp2_aug = h @ [W2 | W2 a2_src | W2 a2_dst] for own nodes,
               AllGather across the 8 cores.
      Phase B2: same edge machinery for layer 2; write z shard.
  - Host: concat shards, invert the node permutation.
"""

import os
import sys
from dataclasses import dataclass

import numpy as np

for _p in ("/opt/trn_rl_repo", "/root/.axon_site/_ro/trn_rl_repo"):
    if os.path.isdir(_p) and _p not in sys.path:
        sys.path.append(_p)

import concourse.bacc as bacc
import concourse.bass as bass
import concourse.mybir as mybir
import concourse.tile as tile
from concourse import bass_utils

F32 = mybir.dt.float32
I16 = mybir.dt.int16

NEG_SLOPE = 0.2
EXP_CLAMP = 30.0


class _Trunc(Exception):
    """Phase-truncation sentinel for KPHASES debugging builds."""


@dataclass(frozen=True)
class GATCfg:
    n_cores: int
    n_pad: int        # padded node count (blocks_total * 128)
    npc: int          # nodes per core
    bpc: int          # blocks per core
    lo_rows: int      # src ids < lo_rows go through the "lo" gather table
    t_lo: int         # tiles of 128 lo-src edges per block
    t_hi: int         # tiles of 128 hi-src edges per block
    in_c: int         # input channels (128)
    hc: int           # heads * hid (256)
    heads: int        # 4
    hid: int          # 64
    out_c: int        # 64
    row1: int         # layer-1 table row width in floats (hc + 64)
    ag_padded: bool   # AllGather into padded 128-wide rows (fallback path)

    @property
    def t_b(self):
        return self.t_lo + self.t_hi


def _wrap_idx(arr):
    """dma_gather index layout: linear i -> (partition i%16, col i//16),
    replicated across the 8 Q7 cores (16-partition pattern tiled to 128)."""
    assert arr.size % 16 == 0
    w = arr.reshape(-1, 16).T  # [16, n/16]
    return np.tile(w, (8, 1))  # [128, n/16]


def prep(x, edge_index, W1, a1_src, a1_dst, b1, W2, a2_src, a2_dst, b2,
         n_cores=8, lo_rows_cap=32768):
    N, IN_C = x.shape
    HEADS, HID = a1_src.shape
    HC = HEADS * HID
    OUT_C = W2.shape[1]

    blk_per_core = -(-N // (128 * n_cores))
    npc = blk_per_core * 128
    n_pad = npc * n_cores
    blocks_total = n_pad // 128
    lo_rows = min(lo_rows_cap, n_pad)

    src = np.asarray(edge_index[0], dtype=np.int64)
    dst = np.asarray(edge_index[1], dtype=np.int64)

    # in-degree incl. self-loop, over padded node set
    deg = np.bincount(dst, minlength=n_pad).astype(np.int64) + 1

    # snake-pack nodes into blocks by descending degree -> balanced block loads
    order = np.argsort(-deg, kind="stable")
    rounds = np.arange(n_pad) // blocks_total
    pos = np.arange(n_pad) % blocks_total
    blk_of_sorted = np.where(rounds % 2 == 0, pos, blocks_total - 1 - pos)
    slot_of_sorted = rounds
    pid_of = np.empty(n_pad, dtype=np.int64)
    pid_of[order] = blk_of_sorted * 128 + slot_of_sorted

    # all edges incl. self-loops for every (padded) node, in permuted space
    ps = np.concatenate([pid_of[src], np.arange(n_pad)])
    pd = np.concatenate([pid_of[dst], np.arange(n_pad)])
    pd_blk = pd >> 7

    is_lo = ps < lo_rows
    # group edges by (block, hi/lo): sort by block*2 + (1-is_lo)
    gkey = pd_blk * 2 + (~is_lo).astype(np.int64)
    eorder = np.argsort(gkey, kind="stable")
    ps_s, pd_s, key_s = ps[eorder], pd[eorder], gkey[eorder]

    cnt = np.bincount(gkey, minlength=blocks_total * 2)
    cnt_lo = cnt[0::2]
    cnt_hi = cnt[1::2]
    t_lo = int(-(-cnt_lo.max() // 128)) if cnt_lo.max() > 0 else 0
    t_hi = int(-(-cnt_hi.max() // 128)) if cnt_hi.max() > 0 else 0
    if t_hi == 0 and lo_rows < n_pad:
        t_hi = 1
    t_b = t_lo + t_hi
    bpc = blk_per_core

    # per-block slot arrays
    slots = blocks_total * t_b * 128
    slot_ps = np.zeros(slots, dtype=np.int64)          # gather idx (pad 0)
    slot_rel = np.full(slots, -1.0, dtype=np.float32)  # dst_rel (pad -1)
    slot_pd = np.zeros(slots, dtype=np.int64)          # dst id   (pad 0)

    ends = np.cumsum(cnt)
    starts = ends - cnt
    # positions of each group's edges within the block's slot array
    grp = key_s
    within = np.arange(len(ps_s)) - starts[grp]
    base = (grp >> 1) * (t_b * 128) + np.where(grp % 2 == 0, 0, t_lo * 128)
    slot_idx = base + within
    slot_ps[slot_idx] = ps_s
    slot_rel[slot_idx] = (pd_s & 127).astype(np.float32)
    slot_pd[slot_idx] = pd_s
    # padding dst ids: keep 0 -> but idx_dst is relative to the OWN core's
    # shard; pad slots use the core's first node (rel id 0) which is valid.

    slot_ps = slot_ps.reshape(n_cores, bpc, t_b * 128)
    slot_rel = slot_rel.reshape(n_cores, bpc, t_b * 128)
    slot_pd = slot_pd.reshape(n_cores, bpc, t_b * 128)

    hi_elems = 0 if t_hi == 0 else 1
    cfg = GATCfg(n_cores=n_cores, n_pad=n_pad, npc=npc, bpc=bpc,
                 lo_rows=lo_rows, t_lo=t_lo, t_hi=t_hi, in_c=IN_C, hc=HC,
                 heads=HEADS, hid=HID, out_c=OUT_C, row1=HC + 64,
                 ag_padded=True)

    # ---- shared (replicated) tensors ----
    xT = np.zeros((IN_C, n_pad), dtype=np.float32)
    xT[:, pid_of[:N]] = np.asarray(x, dtype=np.float32).T

    W1 = np.asarray(W1, np.float32)
    w1s = np.stack([W1[:, h * HID:(h + 1) * HID] @ np.asarray(a1_src, np.float32)[h]
                    for h in range(HEADS)], axis=1)          # [IN_C, H]
    w1d = np.stack([W1[:, h * HID:(h + 1) * HID] @ np.asarray(a1_dst, np.float32)[h]
                    for h in range(HEADS)], axis=1)
    W1a = np.concatenate([W1, w1s, w1d], axis=1)             # [IN_C, HC+8]
    W1a_pad = np.zeros((IN_C, HC + 16), dtype=np.float32)
    W1a_pad[:, :HC + 8] = W1a

    W2 = np.asarray(W2, np.float32)
    w2s = (W2 @ np.asarray(a2_src, np.float32)[0])[:, None]  # [HC, 1]
    w2d = (W2 @ np.asarray(a2_dst, np.float32)[0])[:, None]
    W2a = np.concatenate([W2, w2s, w2d], axis=1)             # [HC, OUT_C+2]
    c2 = OUT_C + 2
    W2s = np.zeros((128, (HC // 128) * c2), dtype=np.float32)
    for j in range(HC // 128):
        W2s[:, j * c2:(j + 1) * c2] = W2a[j * 128:(j + 1) * 128]

    B1 = np.tile(np.asarray(b1, np.float32)[None, :], (128, 1))
    B2 = np.tile(np.asarray(b2, np.float32)[None, :], (128, 1))
    IOTA = np.tile(np.arange(128, dtype=np.float32)[None, :], (128, 1))
    IDN = np.eye(128, dtype=np.float32)

    in_maps = []
    for c in range(n_cores):
        lo_parts, hi_parts, dst_parts = [], [], []
        for b in range(bpc):
            s_ps = slot_ps[c, b]
            s_pd = slot_pd[c, b]
            lo_parts.append(_wrap_idx(s_ps[:t_lo * 128].astype(np.int16)))
            if t_hi:
                hi_parts.append(_wrap_idx(
                    (s_ps[t_lo * 128:] - lo_rows).clip(min=0).astype(np.int16)))
            dst_parts.append(_wrap_idx((s_pd - c * npc).clip(0, npc - 1)
                                       .astype(np.int16)))
        # dst_rel matrix: [128 lanes, bpc*t_b tiles]
        rel = slot_rel[c].reshape(bpc * t_b, 128).T.copy()
        m = {
            "xT": xT, "W1a": W1a_pad, "W2s": W2s, "B1": B1, "B2": B2,
            "IOTA": IOTA, "IDN": IDN,
            "idxlo": np.concatenate(lo_parts, axis=1).astype(np.int16),
            "idxdst": np.concatenate(dst_parts, axis=1).astype(np.int16),
            "dstrel": np.ascontiguousarray(rel),
        }
        if t_hi:
            m["idxhi"] = np.concatenate(hi_parts, axis=1).astype(np.int16)
        in_maps.append(m)

    return cfg, in_maps, pid_of[:N]


def build(cfg: GATCfg):
    level = {"A": 0, "B1": 1, "C": 2, "AG": 3, "full": 4}[
        os.environ.get("KPHASES", "full")]
    P = 128
    HC, H, HID, OC = cfg.hc, cfg.heads, cfg.hid, cfg.out_c
    R1 = cfg.row1
    C2 = OC + 2
    T_LO, T_HI, T_B = cfg.t_lo, cfg.t_hi, cfg.t_b
    BPC, NPC, NPAD = cfg.bpc, cfg.npc, cfg.n_pad
    LO = cfg.lo_rows
    NBLK = NPAD // P
    R2 = 128  # layer-2 table row width (floats)

    nc = bacc.Bacc("TRN2", target_bir_lowering=False, debug=False,
                   num_devices=cfg.n_cores)

    xT_t = nc.dram_tensor("xT", [cfg.in_c, NPAD], F32, kind="ExternalInput")
    W1a_t = nc.dram_tensor("W1a", [cfg.in_c, HC + 16], F32, kind="ExternalInput")
    W2s_t = nc.dram_tensor("W2s", [P, (HC // P) * C2], F32, kind="ExternalInput")
    B1_t = nc.dram_tensor("B1", [P, HC], F32, kind="ExternalInput")
    B2_t = nc.dram_tensor("B2", [P, OC], F32, kind="ExternalInput")
    IOTA_t = nc.dram_tensor("IOTA", [P, P], F32, kind="ExternalInput")
    IDN_t = nc.dram_tensor("IDN", [P, P], F32, kind="ExternalInput")
    idxlo_t = nc.dram_tensor("idxlo", [P, BPC * T_LO * 8], I16, kind="ExternalInput")
    idxhi_t = (nc.dram_tensor("idxhi", [P, BPC * T_HI * 8], I16, kind="ExternalInput")
               if T_HI else None)
    idxdst_t = nc.dram_tensor("idxdst", [P, BPC * T_B * 8], I16, kind="ExternalInput")
    dstrel_t = nc.dram_tensor("dstrel", [P, BPC * T_B], F32, kind="ExternalInput")
    z_t = nc.dram_tensor("z", [NPC, OC], F32, kind="ExternalOutput")
    hdump_t = (nc.dram_tensor("hdump", [NPC, HC], F32, kind="ExternalOutput")
               if os.environ.get("KDEBUG") else None)
    pdump_t = (nc.dram_tensor("pdump", [NPC, HC + H], F32, kind="ExternalOutput")
               if os.environ.get("KDEBUG") else None)

    if True:
      with tile.TileContext(nc) as tc:
        with tc.tile_pool(name="dram", bufs=1, space="DRAM") as dram:
            xp_tab = dram.tile([NPAD + 1, R1], F32)
            al_own = dram.tile([NPC, 64], F32)
            xp2_own = dram.tile([NPC, R2], F32)
            xp2_tab = dram.tile([NPAD, R2], F32)
            al2_own = dram.tile([NPC, 64], F32)

            with tc.tile_pool(name="consts", bufs=1) as consts:
                w1a = consts.tile([P, HC + 16], F32)
                w2s = consts.tile([P, (HC // P) * C2], F32)
                b1t = consts.tile([P, HC], F32)
                b2t = consts.tile([P, OC], F32)
                iota = consts.tile([P, P], F32)
                idn = consts.tile([P, P], F32)
                nc.sync.dma_start(out=w1a[:], in_=W1a_t.ap())
                nc.sync.dma_start(out=w2s[:], in_=W2s_t.ap())
                nc.sync.dma_start(out=b1t[:], in_=B1_t.ap())
                nc.sync.dma_start(out=b2t[:], in_=B2_t.ap())
                nc.sync.dma_start(out=iota[:], in_=IOTA_t.ap())
                nc.sync.dma_start(out=idn[:], in_=IDN_t.ap())

                idxlo = consts.tile([P, BPC * T_LO * 8], I16)
                nc.sync.dma_start(out=idxlo[:], in_=idxlo_t.ap())
                if T_HI:
                    idxhi = consts.tile([P, BPC * T_HI * 8], I16)
                    nc.sync.dma_start(out=idxhi[:], in_=idxhi_t.ap())
                idxdst = consts.tile([P, BPC * T_B * 8], I16)
                nc.sync.dma_start(out=idxdst[:], in_=idxdst_t.ap())
                dstrel = consts.tile([P, BPC * T_B], F32)
                nc.sync.dma_start(out=dstrel[:], in_=dstrel_t.ap())

                h_sb = consts.tile([P, BPC * HC], F32)  # layer-1 out, own nodes

                # ---------------- Phase A ----------------
                CH = min(8, NBLK)  # node tiles per xT load
                with tc.tile_pool(name="pa_x", bufs=2) as pa_x, \
                     tc.tile_pool(name="pa_ps", bufs=2, space="PSUM") as pa_ps, \
                     tc.tile_pool(name="pa_o", bufs=3) as pa_o:
                    for ch0 in range(0, NBLK, CH):
                        cw = min(CH, NBLK - ch0)
                        xt = pa_x.tile([P, CH * P], F32, tag="xt")
                        nc.sync.dma_start(
                            out=xt[:, 0:cw * P],
                            in_=xT_t.ap()[:, ch0 * P:(ch0 + cw) * P])
                        for j in range(cw):
                            t = ch0 + j
                            ps = pa_ps.tile([P, HC + 16], F32, tag="paps")
                            nc.tensor.matmul(out=ps[:], lhsT=xt[:, j * P:(j + 1) * P],
                                             rhs=w1a[:], start=True, stop=True)
                            ot = pa_o.tile([P, HC + 8], F32, tag="pao")
                            nc.any.tensor_copy(out=ot[:], in_=ps[:, 0:HC + 8])
                            nc.sync.dma_start(
                                out=xp_tab[t * P:(t + 1) * P, 0:HC + 8], in_=ot[:])

                # own-shard score table: al_own[:, 0:8] = xp_tab[c*NPC:.., HC:HC+8]
                pid = nc.gpsimd.partition_id()
                row0 = pid * NPC
                nc.gpsimd.dma_start(
                    out=al_own[:, 0:8],
                    in_=xp_tab[bass.ds(row0, NPC), HC:HC + 8])

                # ---------------- Phase B1 ----------------
                with tc.tile_pool(name="b1_sx", bufs=2) as sxp, \
                     tc.tile_pool(name="b1_sd", bufs=2) as sdp, \
                     tc.tile_pool(name="b1_mt", bufs=4) as mtp, \
                     tc.tile_pool(name="b1_rhs", bufs=4) as rhp, \
                     tc.tile_pool(name="b1_sm", bufs=8) as smp, \
                     tc.tile_pool(name="b1_ps", bufs=2, space="PSUM") as psp, \
                     tc.tile_pool(name="b1_hw", bufs=3) as hwp:
                    if level < 2:
                        nc.vector.memset(h_sb[:], 0.0)
                    for b in range(BPC if level >= 1 else 0):
                        sx = sxp.tile([P, T_B, R1], F32, tag="sx")
                        sd = sdp.tile([P, T_B, 64], F32, tag="sd")
                        nc.gpsimd.dma_gather(
                            out_ap=sx[:, 0:T_LO, :],
                            in_ap=xp_tab[0:LO, :],
                            idxs_ap=idxlo[:, b * T_LO * 8:(b + 1) * T_LO * 8],
                            num_idxs=T_LO * P, num_idxs_reg=T_LO * P,
                            elem_size=R1, single_packet=False)
                        if T_HI:
                            nc.gpsimd.dma_gather(
                                out_ap=sx[:, T_LO:T_B, :],
                                in_ap=xp_tab[LO:NPAD, :],
                                idxs_ap=idxhi[:, b * T_HI * 8:(b + 1) * T_HI * 8],
                                num_idxs=T_HI * P, num_idxs_reg=T_HI * P,
                                elem_size=R1, single_packet=False)
                        nc.gpsimd.dma_gather(
                            out_ap=sd[:],
                            in_ap=al_own[:],
                            idxs_ap=idxdst[:, b * T_B * 8:(b + 1) * T_B * 8],
                            num_idxs=T_B * P, num_idxs_reg=T_B * P,
                            elem_size=64, single_packet=False)
                        psb = psp.tile([P, HC + H], F32, tag="psb")
                        for t in range(T_B):
                            gt = b * T_B + t
                            X = sx[:, t, :]
                            mt = mtp.tile([P, P], F32, tag="mt")
                            nc.vector.tensor_scalar(
                                out=mt[:], in0=iota[:],
                                scalar1=dstrel[:, gt:gt + 1], scalar2=None,
                                op0=mybir.AluOpType.is_equal)
                            z4 = smp.tile([P, H], F32, tag="z4")
                            nc.vector.tensor_tensor(
                                out=z4[:], in0=X[:, HC:HC + H],
                                in1=sd[:, t, 4:4 + H], op=mybir.AluOpType.add)
                            zl = smp.tile([P, H], F32, tag="zl")
                            nc.vector.tensor_scalar(
                                out=zl[:], in0=z4[:], scalar1=NEG_SLOPE,
                                scalar2=EXP_CLAMP, op0=mybir.AluOpType.mult,
                                op1=mybir.AluOpType.min)
                            ze = smp.tile([P, H], F32, tag="ze")
                            nc.vector.tensor_scalar(
                                out=ze[:], in0=z4[:], scalar1=EXP_CLAMP,
                                scalar2=None, op0=mybir.AluOpType.min)
                            rt = rhp.tile([P, HC + H], F32, tag="rt")
                            nc.vector.tensor_tensor(
                                out=rt[:, HC:HC + H], in0=ze[:], in1=zl[:],
                                op=mybir.AluOpType.max)
                            nc.scalar.activation(
                                out=rt[:, HC:HC + H], in_=rt[:, HC:HC + H],
                                func=mybir.ActivationFunctionType.Exp)
                            for h in range(H):
                                nc.scalar.mul(
                                    out=rt[:, h * HID:(h + 1) * HID],
                                    in_=X[:, h * HID:(h + 1) * HID],
                                    mul=rt[:, HC + h:HC + h + 1])
                            nc.tensor.matmul(out=psb[:], lhsT=mt[:], rhs=rt[:],
                                             start=(t == 0), stop=(t == T_B - 1))
                        # block epilogue: h = ELU(psum/denom + b1)
                        if pdump_t is not None:
                            pd_sb = hwp.tile([P, HC + H], F32, tag="pdsb")
                            nc.any.tensor_copy(out=pd_sb[:], in_=psb[:])
                            nc.sync.dma_start(
                                out=pdump_t.ap()[b * P:(b + 1) * P, :],
                                in_=pd_sb[:])
                        rec = smp.tile([P, H], F32, tag="rec")
                        nc.vector.reciprocal(out=rec[:], in_=psb[:, HC:HC + H])
                        hb = hwp.tile([P, HC], F32, tag="hb")
                        for h in range(H):
                            nc.scalar.mul(out=hb[:, h * HID:(h + 1) * HID],
                                          in_=psb[:, h * HID:(h + 1) * HID],
                                          mul=rec[:, h:h + 1])
                        nc.vector.tensor_tensor(out=hb[:], in0=hb[:], in1=b1t[:],
                                                op=mybir.AluOpType.add)
                        tn = hwp.tile([P, HC], F32, tag="tn")
                        nc.vector.tensor_scalar(
                            out=tn[:], in0=hb[:], scalar1=0.0, scalar2=None,
                            op0=mybir.AluOpType.min)
                        nc.scalar.activation(out=tn[:], in_=tn[:],
                                             func=mybir.ActivationFunctionType.Exp)
                        tp = hwp.tile([P, HC], F32, tag="tp")
                        nc.vector.tensor_scalar(
                            out=tp[:], in0=hb[:], scalar1=0.0, scalar2=None,
                            op0=mybir.AluOpType.max)
                        nc.vector.tensor_tensor(out=tn[:], in0=tn[:], in1=tp[:],
                                                op=mybir.AluOpType.add)
                        nc.vector.tensor_scalar(
                            out=h_sb[:, b * HC:(b + 1) * HC], in0=tn[:],
                            scalar1=-1.0, scalar2=None, op0=mybir.AluOpType.add)
                        if hdump_t is not None:
                            nc.sync.dma_start(
                                out=hdump_t.ap()[b * P:(b + 1) * P, :],
                                in_=h_sb[:, b * HC:(b + 1) * HC])

                # ---------------- Phase C ----------------
                with tc.tile_pool(name="c_tp", bufs=2, space="PSUM") as ctp, \
                     tc.tile_pool(name="c_ps", bufs=2, space="PSUM") as cps, \
                     tc.tile_pool(name="c_hT", bufs=3) as chp, \
                     tc.tile_pool(name="c_o", bufs=3) as cop:
                    for b in range(BPC if level >= 2 else 0):
                        p2 = cps.tile([P, C2], F32, tag="p2")
                        for j in range(HC // P):
                            pt = ctp.tile([P, P], F32, tag="pt")
                            nc.tensor.transpose(
                                out=pt[:],
                                in_=h_sb[:, b * HC + j * P: b * HC + (j + 1) * P],
                                identity=idn[:])
                            hT = chp.tile([P, P], F32, tag="hT")
                            nc.any.tensor_copy(out=hT[:], in_=pt[:])
                            nc.tensor.matmul(out=p2[:], lhsT=hT[:],
                                             rhs=w2s[:, j * C2:(j + 1) * C2],
                                             start=(j == 0), stop=(j == HC // P - 1))
                        o2 = cop.tile([P, R2], F32, tag="o2")
                        nc.vector.memset(o2[:, C2:R2], 0.0)
                        nc.any.tensor_copy(out=o2[:, 0:C2], in_=p2[:])
                        nc.sync.dma_start(out=xp2_own[b * P:(b + 1) * P, :],
                                          in_=o2[:])

                if level >= 3:
                    nc.gpsimd.collective_compute(
                        "AllGather", mybir.AluOpType.bypass,
                        ins=[xp2_own.opt()],
                        outs=[xp2_tab.opt()],
                        replica_groups=[list(range(cfg.n_cores))])

                    nc.gpsimd.dma_start(
                        out=al2_own[:, 0:2],
                        in_=xp2_tab[bass.ds(row0, NPC), OC:OC + 2])

                # ---------------- Phase B2 ----------------
                with tc.tile_pool(name="b2_sx", bufs=2) as sxp2, \
                     tc.tile_pool(name="b2_sd", bufs=2) as sdp2, \
                     tc.tile_pool(name="b2_mt", bufs=4) as mtp2, \
                     tc.tile_pool(name="b2_rhs", bufs=4) as rhp2, \
                     tc.tile_pool(name="b2_sm", bufs=8) as smp2, \
                     tc.tile_pool(name="b2_ps", bufs=2, space="PSUM") as psp2, \
                     tc.tile_pool(name="b2_z", bufs=3) as zp:
                    for b in range(BPC if level >= 4 else 0):
                        sx = sxp2.tile([P, T_B, R2], F32, tag="sx2")
                        sd = sdp2.tile([P, T_B, 64], F32, tag="sd2")
                        nc.gpsimd.dma_gather(
                            out_ap=sx[:, 0:T_LO, :],
                            in_ap=xp2_tab[0:LO, :],
                            idxs_ap=idxlo[:, b * T_LO * 8:(b + 1) * T_LO * 8],
                            num_idxs=T_LO * P, num_idxs_reg=T_LO * P,
                            elem_size=R2, single_packet=False)
                        if T_HI:
                            nc.gpsimd.dma_gather(
                                out_ap=sx[:, T_LO:T_B, :],
                                in_ap=xp2_tab[LO:NPAD, :],
                                idxs_ap=idxhi[:, b * T_HI * 8:(b + 1) * T_HI * 8],
                                num_idxs=T_HI * P, num_idxs_reg=T_HI * P,
                                elem_size=R2, single_packet=False)
                        nc.gpsimd.dma_gather(
                            out_ap=sd[:],
                            in_ap=al2_own[:],
                            idxs_ap=idxdst[:, b * T_B * 8:(b + 1) * T_B * 8],
                            num_idxs=T_B * P, num_idxs_reg=T_B * P,
                            elem_size=64, single_packet=False)
                        psb = psp2.tile([P, OC + 1], F32, tag="psb2")
                        for t in range(T_B):
                            gt = b * T_B + t
                            X = sx[:, t, :]
                            mt = mtp2.tile([P, P], F32, tag="mt2")
                            nc.vector.tensor_scalar(
                                out=mt[:], in0=iota[:],
                                scalar1=dstrel[:, gt:gt + 1], scalar2=None,
                                op0=mybir.AluOpType.is_equal)
                            z1 = smp2.tile([P, 1], F32, tag="z1")
                            nc.vector.tensor_tensor(
                                out=z1[:], in0=X[:, OC:OC + 1],
                                in1=sd[:, t, 1:2], op=mybir.AluOpType.add)
                            zl = smp2.tile([P, 1], F32, tag="zl2")
                            nc.vector.tensor_scalar(
                                out=zl[:], in0=z1[:], scalar1=NEG_SLOPE,
                                scalar2=EXP_CLAMP, op0=mybir.AluOpType.mult,
                                op1=mybir.AluOpType.min)
                            ze = smp2.tile([P, 1], F32, tag="ze2")
                            nc.vector.tensor_scalar(
                                out=ze[:], in0=z1[:], scalar1=EXP_CLAMP,
                                scalar2=None, op0=mybir.AluOpType.min)
                            rt = rhp2.tile([P, OC + 1], F32, tag="rt2")
                            nc.vector.tensor_tensor(
                                out=rt[:, OC:OC + 1], in0=ze[:], in1=zl[:],
                                op=mybir.AluOpType.max)
                            nc.scalar.activation(
                                out=rt[:, OC:OC + 1], in_=rt[:, OC:OC + 1],
                                func=mybir.ActivationFunctionType.Exp)
                            nc.scalar.mul(out=rt[:, 0:OC], in_=X[:, 0:OC],
                                          mul=rt[:, OC:OC + 1])
                            nc.tensor.matmul(out=psb[:], lhsT=mt[:], rhs=rt[:],
                                             start=(t == 0), stop=(t == T_B - 1))
                        rec = smp2.tile([P, 1], F32, tag="rec2")
                        nc.vector.reciprocal(out=rec[:], in_=psb[:, OC:OC + 1])
                        zb = zp.tile([P, OC], F32, tag="zb")
                        nc.scalar.mul(out=zb[:], in_=psb[:, 0:OC], mul=rec[:])
                        nc.vector.tensor_tensor(out=zb[:], in0=zb[:], in1=b2t[:],
                                                op=mybir.AluOpType.add)
                        nc.sync.dma_start(out=z_t.ap()[b * P:(b + 1) * P, :],
                                          in_=zb[:])

    nc.compile()
    return nc


_CACHE = {}


def _get_built(cfg):
    key = (cfg, os.environ.get("KPHASES", "full"))
    if key not in _CACHE:
        _CACHE[key] = build(cfg)
    return _CACHE[key]


class Runner:
    """Executes the compiled Bass module via PJRT/shard_map with inputs
    pre-sharded per device (no on-device resharding programs)."""

    def __init__(self, nc, n_cores):
        import jax
        from jax.sharding import Mesh, PartitionSpec, NamedSharding
        from jax.experimental.shard_map import shard_map
        from concourse import bass2jax

        bass2jax.install_neuronx_cc_hook()
        self.jax = jax
        self.nc = nc
        self.n_cores = n_cores

        pname = nc.partition_id_tensor.name if nc.partition_id_tensor else None
        in_names, out_names, out_avals = [], [], []
        for alloc in nc.m.functions[0].allocations:
            if not isinstance(alloc, mybir.MemoryLocationSet):
                continue
            name = alloc.memorylocations[0].name
            if alloc.kind == "ExternalInput":
                if name != pname:
                    in_names.append(name)
            elif alloc.kind == "ExternalOutput":
                out_names.append(name)
                out_avals.append(jax.core.ShapedArray(
                    tuple(alloc.tensor_shape), mybir.dt.np(alloc.dtype)))
        self.in_names, self.out_names, self.out_avals = in_names, out_names, out_avals
        all_in = list(in_names) + list(out_names)
        if pname is not None:
            all_in.append(pname)

        def _body(*args):
            operands = list(args)
            if pname is not None:
                operands.append(bass2jax.partition_id_tensor())
            outs = bass2jax._bass_exec_p.bind(
                *operands,
                out_avals=tuple(out_avals),
                in_names=tuple(all_in),
                out_names=tuple(out_names),
                lowering_input_output_aliases=(),
                sim_require_finite=True,
                sim_require_nnan=True,
                nc=nc,
            )
            return tuple(outs)

        self.devices = jax.devices()[:n_cores]
        self.mesh = Mesh(np.asarray(self.devices), ("core",))
        self.sh = NamedSharding(self.mesh, PartitionSpec("core"))
        nspec = (PartitionSpec("core"),)
        self.fn = jax.jit(
            shard_map(_body, mesh=self.mesh,
                      in_specs=nspec * (len(in_names) + len(out_names)),
                      out_specs=nspec * len(out_names), check_rep=False),
            keep_unused=True)
        self.dev_args = None

    def _shard(self, per_core):
        jax = self.jax
        a0 = np.asarray(per_core[0])
        gshape = (self.n_cores * a0.shape[0],) + a0.shape[1:]
        bufs = [jax.device_put(np.asarray(per_core[c]), self.devices[c])
                for c in range(self.n_cores)]
        return jax.make_array_from_single_device_arrays(gshape, self.sh, bufs)

    def set_inputs(self, in_maps):
        args = [self._shard([m[name] for m in in_maps])
                for name in self.in_names]
        for av in self.out_avals:
            z = np.zeros(av.shape, av.dtype)
            args.append(self._shard([z] * self.n_cores))
        self.dev_args = args

    def call(self):
        outs = self.fn(*self.dev_args)
        self.jax.block_until_ready(outs)
        return outs

    def make_k_fn(self, k):
        import jax
        from jax.experimental.shard_map import shard_map
        from jax.sharding import PartitionSpec
        from concourse import bass2jax
        nc = self.nc
        pname = nc.partition_id_tensor.name if nc.partition_id_tensor else None
        all_in = list(self.in_names) + list(self.out_names)
        if pname is not None:
            all_in.append(pname)
        out_avals = self.out_avals

        def _body(*args):
            operands = list(args)
            if pname is not None:
                operands.append(bass2jax.partition_id_tensor())
            outs = None
            for _ in range(k):
                outs = bass2jax._bass_exec_p.bind(
                    *operands,
                    out_avals=tuple(out_avals),
                    in_names=tuple(all_in),
                    out_names=tuple(self.out_names),
                    lowering_input_output_aliases=(),
                    sim_require_finite=True,
                    sim_require_nnan=True,
                    nc=nc,
                )
            return tuple(outs)

        nspec = (PartitionSpec("core"),)
        return jax.jit(
            shard_map(_body, mesh=self.mesh,
                      in_specs=nspec * (len(self.in_names) + len(self.out_names)),
                      out_specs=nspec * len(self.out_names), check_rep=False),
            keep_unused=True)

    def bench(self, k_hi=12, k_lo=2, reps=5):
        import time
        f_lo = self.make_k_fn(k_lo)
        f_hi = self.make_k_fn(k_hi)
        self.jax.block_until_ready(f_lo(*self.dev_args))
        self.jax.block_until_ready(f_hi(*self.dev_args))
        t_lo, t_hi = [], []
        for _ in range(reps):
            t0 = time.perf_counter()
            self.jax.block_until_ready(f_lo(*self.dev_args))
            t_lo.append(time.perf_counter() - t0)
            t0 = time.perf_counter()
            self.jax.block_until_ready(f_hi(*self.dev_args))
            t_hi.append(time.perf_counter() - t0)
        per_iter = (min(t_hi) - min(t_lo)) / (k_hi - k_lo)
        return per_iter, min(t_lo), min(t_hi)

    def run(self, in_maps):
        self.set_inputs(in_maps)
        outs = self.call()
        res = []
        for c in range(self.n_cores):
            d = {}
            for i, name in enumerate(self.out_names):
                g = np.asarray(outs[i])
                n0 = self.out_avals[i].shape[0]
                d[name] = g.reshape(self.n_cores, n0, *self.out_avals[i].shape[1:])[c]
            res.append(d)
        return res


_RUNNERS = {}


def _get_runner(cfg, nc):
    key = id(nc)
    if key not in _RUNNERS:
        _RUNNERS[key] = Runner(nc, cfg.n_cores)
    return _RUNNERS[key]


def kernel(x, edge_index, W1, a1_src, a1_dst, b1, W2, a2_src, a2_dst, b2):
    x = np.asarray(x)
    cfg, in_maps, pid_of = prep(x, edge_index, W1, a1_src, a1_dst, b1,
                                W2, a2_src, a2_dst, b2)
    nc = _get_built(cfg)
    runner = _get_runner(cfg, nc)
    results = runner.run(in_maps)
    z_full = np.concatenate([results[c]["z"] for c in range(cfg.n_cores)],
                            axis=0)
    return np.ascontiguousarray(z_full[pid_of]).astype(np.float32)
